# revision 1
# baseline (speedup 1.0000x reference)
"""Multi-head causal attention on 8 TRN2 NeuronCores.

Sharding: tensor-parallel over heads — 16 heads / 8 cores = 2 heads per core.
Each core computes q/k/v projections for its 2 heads (column-sharded QKV
weights), causal attention for those heads over both batch elements, and the
row-sharded slice of the output projection, producing a full-shape partial
output.  Host sums the 8 partials and adds bo + bv @ Wo.T (the per-head value
bias commutes through the output projection because attention rows sum to 1).

All matmuls run in float32r (full-rate fp32 on the PE array, ~1e-4 rel).
Layout/structure choices:
  - x is passed transposed (E-major) so QKV matmuls need no on-device
    transposes; weights are host-transposed likewise
  - scores are computed transposed [k, q] so the attn @ v matmul's operands
    arrive in exactly the layout the PE wants; softmax denominators come from
    a ones-column matmul accumulated alongside
  - diagonal k-tiles only compute/accumulate their causally valid column
    sub-range (exact: diagonal tiles are last in each k-loop)
  - output projection is fused into the attention q-tile loop; evacuation
    copies alternate between DVE and ACT; PSUM tags are budgeted to 8 banks
    (proj/out-proj 2, scores 4, attn accumulator 1, denominators 1)
"""

import sys

if "/opt/trn_rl_repo" not in sys.path:
    sys.path.insert(0, "/opt/trn_rl_repo")

import numpy as np

import concourse.bass as bass  # noqa: F401  (engine namespaces live on nc)
import concourse.tile as tile
from concourse import bacc, mybir
from concourse.bass_utils import run_bass_kernel_spmd

F32 = mybir.dt.float32
F32R = mybir.dt.float32r
AF = mybir.ActivationFunctionType
ALU = mybir.AluOpType

B, S, E = 2, 2048, 2048
H, D = 16, 128
NCORES = 8
HPC = H // NCORES          # heads per core = 2
M = HPC * D                # local channels per core = 256
EO = E // 128              # 16 contraction chunks
XT = 256                   # token-tile width for projections
NT = S // XT               # 8 token tiles per batch
QT = 512                   # q-tile width for attention
NQT = S // QT              # 4 q-tiles
ET = 512                   # e-tile width for out-projection
SCALE = 1.0 / float(np.sqrt(D))
MASK_BIAS = -30.0


def build_nc():
    nc = bacc.Bacc(trn_type="TRN2", target_bir_lowering=False, num_swdge_queues=4)

    xT = nc.declare_dram_parameter("xT", [B, E, S], F32, isOutput=False)
    wq = nc.declare_dram_parameter("wq", [E, M], F32, isOutput=False)
    wk = nc.declare_dram_parameter("wk", [E, M], F32, isOutput=False)
    wv = nc.declare_dram_parameter("wv", [E, M], F32, isOutput=False)
    wo = nc.declare_dram_parameter("wo", [M, E], F32, isOutput=False)
    bq = nc.declare_dram_parameter("bq", [128, HPC], F32, isOutput=False)
    bk = nc.declare_dram_parameter("bk", [128, HPC], F32, isOutput=False)
    tb = nc.declare_dram_parameter("tb", [128, 128], F32, isOutput=False)
    on = nc.declare_dram_parameter("on", [128, 1], F32, isOutput=False)
    o = nc.declare_dram_parameter("o", [B, S, E], F32, isOutput=True)

    with tile.TileContext(nc) as tc:
        _body(tc, nc, xT, wq, wk, wv, wo, bq, bk, tb, on, o)
    nc.compile()
    return nc


def _body(tc, nc, xT, wq, wk, wv, wo, bq, bk, tb, on, o):
    from contextlib import ExitStack

    ctx = ExitStack()
    with ctx:
        wpool = ctx.enter_context(tc.tile_pool(name="w", bufs=1))
        xpool = ctx.enter_context(tc.tile_pool(name="x", bufs=2))
        qkv = ctx.enter_context(tc.tile_pool(name="qkv", bufs=1))
        epool = ctx.enter_context(tc.tile_pool(name="e", bufs=6))
        otp = ctx.enter_context(tc.tile_pool(name="ot", bufs=1))
        osp = ctx.enter_context(tc.tile_pool(name="os", bufs=8))
        rp = ctx.enter_context(tc.tile_pool(name="r", bufs=2))
        psA = ctx.enter_context(tc.tile_pool(name="psA", bufs=2, space="PSUM"))
        psC = ctx.enter_context(tc.tile_pool(name="psC", bufs=4, space="PSUM"))
        psB1 = ctx.enter_context(tc.tile_pool(name="psB1", bufs=1, space="PSUM"))

        # ---- weights / constants (once) ----
        wq_sb = wpool.tile([128, EO, M], F32R, tag="wq")
        wk_sb = wpool.tile([128, EO, M], F32R, tag="wk")
        wv_sb = wpool.tile([128, EO, M], F32R, tag="wv")
        wo_sb = wpool.tile([128, HPC, E], F32R, tag="wo")
        on_sb = wpool.tile([128, 1], F32R, tag="on")
        nc.gpsimd.dma_start(on_sb[:], on[:])
        # Warm the PE (HAM clock gate) with tiny matmuls while x0/weights load.
        warm_rhs = rp.tile([128, 512], F32, tag="rb")
        nc.vector.memset(warm_rhs[:], 0.0)
        warm = psC.tile([128, 512], F32, tag="sc")
        for _ in range(24):
            nc.tensor.matmul(warm[:1, :], on_sb[:], warm_rhs[:].bitcast(F32R),
                             start=True, stop=True)
        # x tile 0 and wq stream in interleaved chunks so the first projection
        # matmuls start as soon as their first contraction chunks land; wo is
        # not needed until attention output, so it loads last.
        x_first = xpool.tile([128, EO, XT], F32R, tag="x")
        _xr0 = xT[0].rearrange("(eo p) s -> p eo s", p=128)
        _wqr = wq.rearrange("(eo p) m -> p eo m", p=128)
        for lo, hi in ((0, 4), (4, 8), (8, 16)):
            nc.gpsimd.dma_start(x_first[:, lo:hi], _xr0[:, lo:hi, 0:XT])
            nc.gpsimd.dma_start(wq_sb[:, lo:hi], _wqr[:, lo:hi])
        nc.gpsimd.dma_start(wk_sb[:], wk.rearrange("(eo p) m -> p eo m", p=128))
        nc.gpsimd.dma_start(wv_sb[:], wv.rearrange("(eo p) m -> p eo m", p=128))
        x_second = xpool.tile([128, EO, XT], F32R, tag="x")
        nc.gpsimd.dma_start(
            x_second[:], xT[0].rearrange("(eo p) s -> p eo s", p=128)[:, :, XT:2 * XT]
        )
        nc.gpsimd.dma_start(wo_sb[:], wo.rearrange("(h p) e -> p h e", p=128))
        bq_sb = wpool.tile([128, HPC], F32, tag="bq")
        bk_sb = wpool.tile([128, HPC], F32, tag="bk")
        nc.sync.dma_start(bq_sb[:], bq[:])
        nc.sync.dma_start(bk_sb[:], bk[:])
        tb_sb = wpool.tile([128, 128], F32, tag="tb")
        nc.sync.dma_start(tb_sb[:], tb[:])


        for b in range(B):
            # ---- Q/K/V projections for batch b ----
            qT_sb = qkv.tile([128, HPC, S], F32R, tag="qT")
            kT_sb = qkv.tile([128, HPC, S], F32R, tag="kT")
            v_sb = qkv.tile([128, S // 128, M], F32R, tag="v")
            oT_sb = otp.tile([128, HPC, S], F32R, tag="oT")
            qT_h = [qT_sb[:, h] for h in range(HPC)]
            kT_h = [kT_sb[:, h] for h in range(HPC)]
            v_h = [v_sb[:, :, h * D:(h + 1) * D] for h in range(HPC)]
            oT_h = [oT_sb[:, h] for h in range(HPC)]
            xTb = xT[b].rearrange("(eo p) s -> p eo s", p=128)

            def proj_tile(t, x_t):
                for h in range(HPC):
                    for w_sb, dsts, bias, scl in (
                        (wq_sb, qT_h, bq_sb, SCALE),
                        (wk_sb, kT_h, bk_sb, 1.0),
                    ):
                        ps = psA.tile([128, 512], F32, tag="qkv")
                        for eo in range(EO):
                            nc.tensor.matmul(
                                ps[:, :XT],
                                w_sb[:, eo, h * D:(h + 1) * D],
                                x_t[:, eo, :],
                                start=(eo == 0),
                                stop=(eo == EO - 1),
                            )
                        nc.scalar.activation(
                            dsts[h][:, t * XT:(t + 1) * XT],
                            ps[:, :XT],
                            AF.Identity,
                            bias=bias[:, h:h + 1],
                            scale=scl,
                        )
                for st in range(XT // 128):
                    ps = psA.tile([128, 512], F32, tag="qkv")
                    for eo in range(EO):
                        nc.tensor.matmul(
                            ps[:, :M],
                            x_t[:, eo, st * 128:(st + 1) * 128],
                            wv_sb[:, eo, :],
                            start=(eo == 0),
                            stop=(eo == EO - 1),
                        )
                    nc.vector.tensor_copy(
                        v_sb[:, t * (XT // 128) + st, :], ps[:, :M]
                    )

            def attn_qtile(h, qt):
                q_rhs = qT_h[h][:, qt * QT:(qt + 1) * QT]
                ut = psB1.tile([128, 512], F32, tag="ut")
                sums = psB1.tile([1, 512], F32, tag="sums")
                nkt = (qt + 1) * (QT // 128)
                for kt in range(nkt):
                    jj = kt - qt * (QT // 128)
                    # columns < jj*128 of this k-tile's block are causally
                    # masked; diagonal tiles come last in the k-loop, so
                    # accumulating only the valid sub-range is exact.
                    lo = max(jj, 0) * 128
                    sc = psC.tile([128, 512], F32, tag="sc")
                    nc.tensor.matmul(
                        sc[:, lo:],
                        kT_h[h][:, kt * 128:(kt + 1) * 128],
                        q_rhs[:, lo:],
                        start=True,
                        stop=True,
                    )
                    e = epool.tile([128, 512], F32R, tag="e")
                    if jj >= 0:
                        nc.vector.tensor_tensor(
                            sc[:, jj * 128:(jj + 1) * 128],
                            sc[:, jj * 128:(jj + 1) * 128],
                            tb_sb[:],
                            ALU.add,
                        )
                        if jj > 0:
                            nc.vector.memset(e[:, :lo].bitcast(F32), 0.0)
                        nc.scalar.activation(e[:, lo:], sc[:, lo:], AF.Exp)
                    else:
                        nc.scalar.activation(e[:], sc[:], AF.Exp)
                    nc.tensor.matmul(
                        ut[:, lo:],
                        v_h[h][:, kt, :],
                        e[:, lo:],
                        start=(kt == 0),
                        stop=(kt == nkt - 1),
                    )
                    nc.tensor.matmul(
                        sums[:, lo:],
                        on_sb[:],
                        e[:, lo:],
                        start=(kt == 0),
                        stop=(kt == nkt - 1),
                    )
                rec = rp.tile([1, 512], F32, tag="rec")
                nc.vector.reciprocal(rec[:], sums[:])
                rb = rp.tile([128, 512], F32, tag="rb")
                nc.gpsimd.partition_broadcast(rb[:], rec[:])
                nc.vector.tensor_tensor(
                    oT_h[h][:, qt * QT:(qt + 1) * QT], ut[:], rb[:], ALU.mult
                )

            def out_proj(qt):
                for qi4 in range(QT // 128):
                    qi = qt * (QT // 128) + qi4
                    for et in range(E // ET):
                        ps = psA.tile([128, 512], F32, tag="qkv")
                        for h in range(HPC):
                            nc.tensor.matmul(
                                ps[:],
                                oT_h[h][:, qi * 128:(qi + 1) * 128],
                                wo_sb[:, h, et * ET:(et + 1) * ET],
                                start=(h == 0),
                                stop=(h == HPC - 1),
                            )
                        osb = osp.tile([128, 512], F32, tag="osb")
                        if (qi * (E // ET) + et) % 2 == 0:
                            nc.vector.tensor_copy(osb[:], ps[:])
                        else:
                            nc.scalar.copy(osb[:], ps[:])
                        nc.sync.dma_start(
                            o[b, qi * 128:(qi + 1) * 128, et * ET:(et + 1) * ET],
                            osb[:],
                        )

            for t in range(NT):
                if b == 0 and t == 0:
                    x_t = x_first
                elif b == 0 and t == 1:
                    x_t = x_second
                else:
                    x_t = xpool.tile([128, EO, XT], F32R, tag="x")
                    nc.gpsimd.dma_start(x_t[:], xTb[:, :, t * XT:(t + 1) * XT])
                proj_tile(t, x_t)
            qts = list(range(NQT)) if b == 0 else list(reversed(range(NQT)))
            for qt in qts:
                attn_qtile(0, qt)
                attn_qtile(1, qt)
                out_proj(qt)


_NC_CACHE = None


def _get_nc():
    global _NC_CACHE
    if _NC_CACHE is None:
        _NC_CACHE = build_nc()
    return _NC_CACHE


def _prep_inputs(x, Wq, bq, Wk, bk, Wv, bv, Wo, bo):
    x = np.ascontiguousarray(np.asarray(x, dtype=np.float32))
    xT = np.ascontiguousarray(x.transpose(0, 2, 1))
    tb_np = np.where(
        np.arange(128)[:, None] <= np.arange(128)[None, :], 0.0, MASK_BIAS
    ).astype(np.float32)
    on_np = np.ones((128, 1), dtype=np.float32)
    in_maps = []
    for c in range(NCORES):
        sl = slice(c * M, (c + 1) * M)
        in_maps.append({
            "xT": xT,
            "wq": np.ascontiguousarray(np.asarray(Wq)[sl, :].T.astype(np.float32)),
            "wk": np.ascontiguousarray(np.asarray(Wk)[sl, :].T.astype(np.float32)),
            "wv": np.ascontiguousarray(np.asarray(Wv)[sl, :].T.astype(np.float32)),
            "wo": np.ascontiguousarray(np.asarray(Wo)[:, sl].T.astype(np.float32)),
            "bq": np.ascontiguousarray(
                (np.asarray(bq)[sl].astype(np.float32) * SCALE).reshape(HPC, 128).T
            ),
            "bk": np.ascontiguousarray(
                np.asarray(bk)[sl].astype(np.float32).reshape(HPC, 128).T
            ),
            "tb": tb_np,
            "on": on_np,
        })
    return in_maps


def run(inputs, trace=False):
    in_maps = _prep_inputs(
        inputs["x"], inputs["Wq"], inputs["bq"], inputs["Wk"], inputs["bk"],
        inputs["Wv"], inputs["bv"], inputs["Wo"], inputs["bo"],
    )
    nc = _get_nc()
    res = run_bass_kernel_spmd(nc, in_maps, list(range(NCORES)), trace=trace)
    acc = np.zeros((B, S, E), dtype=np.float64)
    for r in res.results:
        acc += r["o"].astype(np.float64)
    acc += np.asarray(inputs["bo"], dtype=np.float64)[None, None, :]
    acc += (np.asarray(inputs["bv"], dtype=np.float64)
            @ np.asarray(inputs["Wo"], dtype=np.float64).T)[None, None, :]
    return acc.astype(np.float32), res


def kernel(**inputs):
    out, _ = run(inputs, trace=False)
    return out



# revision 59
# speedup vs baseline: 2.0239x; 2.0239x over previous
"""Multi-head causal attention on 8 TRN2 NeuronCores — fp8 DoubleRow edition.

Sharding: tensor-parallel over heads (2 heads/core).  Each core computes its
q/k/v projections, causal attention and the row-sharded out-projection slice,
producing a full-shape bf16 partial summed on host (+ bo + bv@Wo.T, which
commutes because softmax rows sum to 1).

Speed levers vs the fp32r baseline:
  - all heavy matmuls in fp8e4 with MatmulPerfMode.DoubleRow (2 K-chunks per
    instruction at 0.5 cycles/output-col): QKV projections pair eo-chunks,
    attn@v pairs k-tiles, out-proj pairs the two heads, softmax denominators
    use a [128,2,16] ones lhsT, and scores zero-pad the second K-chunk (the
    pad is free: cost scales only with output columns)
  - causal masking via gpsimd affine_select zeroing e-tiles after exp
    (no mask-bias adds on DVE); diagonal pairs use two dedicated e-tiles
    whose below-diagonal prefixes are zeroed once
  - rows 0-127 of each batch redone in bf16 end-to-end (their peaked softmax
    amplifies fp8 error); fp8 results for those rows are discarded
  - bf16 output partials (half the store traffic), host sums in float64

Scheduling (the cost model serializes each engine's queue in emission order,
so producer/consumer interleaving is explicit):
  - attention ut/sums matmuls lag their pair's exp by two pairs; ut is
    copied to SBUF immediately so the single ut PSUM bank turns around fast
  - projections for batch b and out-proj tiles for q-tile qt-1 drain into
    the attention pair slots as weighted "fillers" to keep the in-order PE
    fed while ACT streams exps
  - out-proj evacuation is the scarce resource (PSUM->SBUF is ACT/DVE only):
    it rides inside both batches' attention windows DVE-only on the psA
    banks (sharing the scores' psC banks would serialize the exp pipeline),
    with only the last chunk as an engine-alternating tail
  - all DMA shares one ~360GB/s track in the model: loads are emitted
    strictly first-need-first
TimelineSim: 183.7us vs 371.7us fp32r baseline; max rel err 7.8e-3 on HW.
"""

import sys

if "/opt/trn_rl_repo" not in sys.path:
    sys.path.insert(0, "/opt/trn_rl_repo")

import numpy as np
import ml_dtypes

import concourse.bass as bass  # noqa: F401
import concourse.tile as tile
from concourse import bacc, mybir
from concourse.bass_utils import run_bass_kernel_spmd

F32 = mybir.dt.float32
BF16 = mybir.dt.bfloat16
F8 = mybir.dt.float8e4
AF = mybir.ActivationFunctionType
ALU = mybir.AluOpType
DR = mybir.MatmulPerfMode.DoubleRow

B, S, E = 2, 2048, 2048
H, D = 16, 128
NCORES = 8
HPC = H // NCORES          # heads per core = 2
M = HPC * D                # local out channels = 256
EO = E // 128              # 16 contraction chunks
XT = 512                   # token-tile width for projections
NT = S // XT               # 4 x-tiles per batch
QT = 512                   # q-tile width
NQT = S // QT
SCALE = 1.0 / float(np.sqrt(D))
RED = 128                  # tokens redone in bf16 per batch


def build_nc():
    nc = bacc.Bacc(trn_type="TRN2", target_bir_lowering=False, num_swdge_queues=4)

    x8 = nc.declare_dram_parameter("x8", [B, NT, 128, EO, XT], F8, isOutput=False)
    xw = nc.declare_dram_parameter("xw", [B, 128, EO, RED], BF16, isOutput=False)
    wq8 = nc.declare_dram_parameter("wq8", [128, EO, M], F8, isOutput=False)
    wk8 = nc.declare_dram_parameter("wk8", [128, EO, M], F8, isOutput=False)
    wv8 = nc.declare_dram_parameter("wv8", [128, EO, M], F8, isOutput=False)
    wo8 = nc.declare_dram_parameter("wo8", [128, HPC, E], F8, isOutput=False)
    wqw = nc.declare_dram_parameter("wqw", [128, EO, M], BF16, isOutput=False)
    wkw = nc.declare_dram_parameter("wkw", [128, EO, M], BF16, isOutput=False)
    wvw = nc.declare_dram_parameter("wvw", [128, EO, M], BF16, isOutput=False)
    wow = nc.declare_dram_parameter("wow", [128, HPC, E], BF16, isOutput=False)
    bq = nc.declare_dram_parameter("bq", [128, HPC], F32, isOutput=False)
    bk = nc.declare_dram_parameter("bk", [128, HPC], F32, isOutput=False)
    on8 = nc.declare_dram_parameter("on8", [128, 2, 16], F8, isOutput=False)
    o = nc.declare_dram_parameter("o", [B, S, E], BF16, isOutput=True)

    with tile.TileContext(nc) as tc:
        _body(tc, nc, x8, xw, wq8, wk8, wv8, wo8, wqw, wkw, wvw, wow,
              bq, bk, on8, o)
    nc.compile()
    return nc


def _body(tc, nc, x8, xw, wq8, wk8, wv8, wo8, wqw, wkw, wvw, wow,
          bq, bk, on8, o):
    from contextlib import ExitStack

    ctx = ExitStack()
    with ctx:
        wp = ctx.enter_context(tc.tile_pool(name="w", bufs=1))
        ded = ctx.enter_context(tc.tile_pool(name="ded", bufs=1))
        xp = ctx.enter_context(tc.tile_pool(name="x", bufs=4))
        xwp = ctx.enter_context(tc.tile_pool(name="xw", bufs=2))
        qkv = ctx.enter_context(tc.tile_pool(name="qkv", bufs=2))
        rd = ctx.enter_context(tc.tile_pool(name="rd", bufs=2))
        ep = ctx.enter_context(tc.tile_pool(name="e", bufs=6))
        rp = ctx.enter_context(tc.tile_pool(name="r", bufs=2))
        utp = ctx.enter_context(tc.tile_pool(name="uts", bufs=2))
        osp = ctx.enter_context(tc.tile_pool(name="os", bufs=12))
        psA = ctx.enter_context(tc.tile_pool(name="psA", bufs=2, space="PSUM"))
        psC = ctx.enter_context(tc.tile_pool(name="psC", bufs=2, space="PSUM"))
        psU = ctx.enter_context(tc.tile_pool(name="psU", bufs=1, space="PSUM"))
        psS = ctx.enter_context(tc.tile_pool(name="psS", bufs=1, space="PSUM"))

        # ---- constants & weights ----
        on8_sb = wp.tile([128, 2, 16], F8, tag="on8")
        nc.sync.dma_start(on8_sb[:], on8[:])
        bq_sb = wp.tile([128, HPC], F32, tag="bq")
        bk_sb = wp.tile([128, HPC], F32, tag="bk")

        # warmup rhs: a zeroed fp8 [128, 2, 512] tile
        ew = ded.tile([128, 2, 512], F8, tag="ew")
        nc.vector.memset(ew[:].bitcast(F32), 0.0)
        warm = psS.tile([16, 512], F32, tag="sm")
        for _ in range(24):
            nc.tensor.matmul(warm[:], on8_sb[:], ew[:], start=True, stop=True,
                             perf_mode=DR)

        # Early loads spread across all three DMA queues so the first
        # projection groups start ~4us in.  gpsimd (SWDGE) triggers cost the
        # Pool engine ~2us each, so latency-critical early tiles ride the
        # HWDGE queues (SP idle, ACT idle at start) instead.
        x_t00 = xp.tile([128, EO, XT], F8, tag="x")
        wq8_sb = wp.tile([128, EO, M], F8, tag="wq8")
        wk8_sb = wp.tile([128, EO, M], F8, tag="wk8")
        wv8_sb = wp.tile([128, EO, M], F8, tag="wv8")
        b0_x = [x_t00]
        for t in range(1, NT):
            x_t = xp.tile([128, EO, XT], F8, tag="x", name=f"x0_{t}")
            b0_x.append(x_t)
        # All transfers share one DMA track, so emission order ~= arrival
        # order: strictly first-need first.
        nc.gpsimd.dma_start(wq8_sb[:], wq8[:])
        nc.scalar.dma_start(x_t00[:, 0:8], x8[0, 0, :, 0:8])
        nc.sync.dma_start(x_t00[:, 8:16], x8[0, 0, :, 8:16])
        nc.scalar.dma_start(wk8_sb[:], wk8[:])
        nc.gpsimd.dma_start(wv8_sb[:], wv8[:])
        nc.scalar.dma_start(b0_x[1][:], x8[0, 1])
        nc.sync.dma_start(bq_sb[:], bq[:])
        nc.sync.dma_start(bk_sb[:], bk[:])
        wqw_sb = wp.tile([128, EO, M], BF16, tag="wqw")
        wkw_sb = wp.tile([128, EO, M], BF16, tag="wkw")
        wvw_sb = wp.tile([128, EO, M], BF16, tag="wvw")
        wow_sb = wp.tile([128, HPC, E], BF16, tag="wow")
        xw_sb = [xwp.tile([128, EO, RED], BF16, tag="xw", name=f"xw{i}")
                 for i in range(B)]
        nc.sync.dma_start(wqw_sb[:], wqw[:])
        nc.scalar.dma_start(wkw_sb[:], wkw[:])
        nc.gpsimd.dma_start(b0_x[2][:], x8[0, 2])
        nc.sync.dma_start(wvw_sb[:], wvw[:])
        nc.scalar.dma_start(xw_sb[0][:], xw[0])
        nc.gpsimd.dma_start(b0_x[3][:], x8[0, 3])
        wo8_sb = wp.tile([128, HPC, E], F8, tag="wo8")
        nc.gpsimd.dma_start(wo8_sb[:], wo8[:])
        nc.sync.dma_start(wow_sb[:], wow[:])
        nc.scalar.dma_start(xw_sb[1][:], xw[1])

        # bf16 ones column for redo denominators
        onw = wp.tile([128, 1], BF16, tag="onw")
        nc.vector.memset(onw[:], 1.0)

        # dedicated diagonal e-tiles: eA holds k-pair (jj=0,1), eB (jj=2,3);
        # double-buffered so consecutive (h, qt) iterations don't serialize
        # on them.  Fully-masked prefixes are zeroed once here; exp only
        # writes the causally live column ranges.
        eAs, eBs = [], []
        for i in range(2):
            eA_ = ded.tile([128, 2, 512], F8, tag=f"eA{i}", name=f"eA{i}")
            eB_ = ded.tile([128, 2, 512], F8, tag=f"eB{i}", name=f"eB{i}")
            nc.vector.memset(eA_[:, 1, 0:128].bitcast(F32), 0.0)
            nc.vector.memset(eB_[:, 0, 0:256].bitcast(F32), 0.0)
            nc.vector.memset(eB_[:, 1, 0:384].bitcast(F32), 0.0)
            eAs.append(eA_); eBs.append(eB_)
        diag_flip = [0]

        # ---- per-batch tiles (double buffered) ----
        qT8s, kT8s, v8s, oT8s = [], [], [], []
        rds = []
        for b in range(B):
            qT8 = qkv.tile([128, 2, HPC, S], F8, tag="qT8")
            kT8 = qkv.tile([128, 2, HPC, S], F8, tag="kT8")
            # zero-pad second K-chunk (lets scores run DoubleRow; the pad
            # contributes nothing and costs nothing — DR charges by output
            # columns)
            nc.vector.memset(qT8[:, 1].bitcast(F32), 0.0)
            nc.vector.memset(kT8[:, 1].bitcast(F32), 0.0)
            v8 = qkv.tile([128, S // 128, M], F8, tag="v8")
            oT8 = qkv.tile([128, HPC, S], F8, tag="oT8")
            qT8s.append(qT8); kT8s.append(kT8); v8s.append(v8); oT8s.append(oT8)
            rds.append({
                "qr": rd.tile([128, HPC, RED], BF16, tag="qr", name=f"qr{b}"),
                "kr": rd.tile([128, HPC, RED], BF16, tag="kr", name=f"kr{b}"),
                "vr": rd.tile([128, M], BF16, tag="vr", name=f"vr{b}"),
                "oTr": rd.tile([128, HPC, RED], BF16, tag="oTr",
                               name=f"oTr{b}"),
            })

        def redo_proj(b, sink):
            x_t = xw_sb[b]
            r = rds[b]
            for h in range(HPC):
                for ww_, dst, bias, scl in (
                    (wqw_sb, r["qr"], bq_sb, SCALE),
                    (wkw_sb, r["kr"], bk_sb, 1.0),
                ):
                    def grp(h=h, ww_=ww_, dst=dst, bias=bias, scl=scl):
                        ps = psA.tile([128, 512], F32, tag="pj")
                        for eo in range(EO):
                            nc.tensor.matmul(
                                ps[:, :RED],
                                ww_[:, eo, h * D:(h + 1) * D],
                                x_t[:, eo, :],
                                start=(eo == 0), stop=(eo == EO - 1),
                            )
                        nc.scalar.activation(
                            dst[:, h, :], ps[:, :RED], AF.Identity,
                            bias=bias[:, h:h + 1], scale=scl,
                        )
                    sink(grp)

            def grpv():
                ps = psA.tile([128, 512], F32, tag="pj")
                for eo in range(EO):
                    nc.tensor.matmul(
                        ps[:, :M], x_t[:, eo, :], wvw_sb[:, eo, :],
                        start=(eo == 0), stop=(eo == EO - 1),
                    )
                nc.vector.tensor_copy(r["vr"][:], ps[:, :M])
            sink(grpv)

        def redo_attn(b):
            r = rds[b]
            for h in range(HPC):
                sc = psC.tile([128, 2, 512], F32, tag="sc")
                nc.tensor.matmul(sc[:, 0, :RED], r["kr"][:, h, :],
                                 r["qr"][:, h, :], start=True, stop=True)
                er = rd.tile([128, RED], BF16, tag="er")
                nc.scalar.activation(er[:], sc[:, 0, :RED], AF.Exp)
                # causal mask: keep where col >= partition
                nc.gpsimd.affine_select(er[:], er[:], [[1, RED]], ALU.is_ge,
                                        0.0, base=0, channel_multiplier=-1)
                ut = psU.tile([128, 512], F32, tag="ut")
                nc.tensor.matmul(ut[:, :RED], r["vr"][:, h * D:(h + 1) * D],
                                 er[:], start=True, stop=True)
                sm = psS.tile([16, 512], F32, tag="sm")
                nc.tensor.matmul(sm[:1, :RED], onw[:], er[:],
                                 start=True, stop=True)
                ut_s = utp.tile([128, 512], F32, tag="uts")
                nc.vector.tensor_copy(ut_s[:, :RED], ut[:, :RED])
                rec = rp.tile([1, 512], F32, tag="rec")
                nc.vector.reciprocal(rec[:, :RED], sm[:1, :RED])
                rb = rp.tile([128, 512], F32, tag="rb")
                nc.gpsimd.partition_broadcast(rb[:, :RED], rec[:, :RED])
                nc.vector.tensor_tensor(r["oTr"][:, h, :], ut_s[:, :RED],
                                        rb[:, :RED], ALU.mult)

        def proj_tile(b, t, x_t, sink):
            qT8, kT8, v8 = qT8s[b], kT8s[b], v8s[b]
            for h in range(HPC):
                for w8_, dst, bias, scl in (
                    (wq8_sb, qT8, bq_sb, SCALE),
                    (wk8_sb, kT8, bk_sb, 1.0),
                ):
                    cell = {}

                    def grp_a(h=h, w8_=w8_, cell=cell):
                        ps = psA.tile([128, 512], F32, tag="pj")
                        cell["ps"] = ps
                        for j in range(EO // 4):
                            nc.tensor.matmul(
                                ps[:],
                                w8_[:, 2 * j:2 * j + 2, h * D:(h + 1) * D],
                                x_t[:, 2 * j:2 * j + 2, :],
                                start=(j == 0), stop=False, perf_mode=DR,
                            )

                    def grp_b(h=h, w8_=w8_, dst=dst, bias=bias, scl=scl,
                              cell=cell):
                        ps = cell["ps"]
                        for j in range(EO // 4, EO // 2):
                            nc.tensor.matmul(
                                ps[:],
                                w8_[:, 2 * j:2 * j + 2, h * D:(h + 1) * D],
                                x_t[:, 2 * j:2 * j + 2, :],
                                start=False, stop=(j == EO // 2 - 1),
                                perf_mode=DR,
                            )
                        # scale+bias evac on DVE (ACT is saturated by exp)
                        if scl != 1.0:
                            nc.vector.tensor_scalar(
                                dst[:, 0, h, t * XT:(t + 1) * XT], ps[:],
                                scl, bias[:, h:h + 1], ALU.mult, ALU.add)
                        else:
                            nc.vector.tensor_scalar(
                                dst[:, 0, h, t * XT:(t + 1) * XT], ps[:],
                                bias[:, h:h + 1], None, ALU.add)
                    sink(grp_a)
                    sink(grp_b)
            for st in range(XT // 128):
                def grpv(st=st):
                    ps = psA.tile([128, 512], F32, tag="pj")
                    for j in range(EO // 2):
                        nc.tensor.matmul(
                            ps[:, :M],
                            x_t[:, 2 * j:2 * j + 2, st * 128:(st + 1) * 128],
                            wv8_sb[:, 2 * j:2 * j + 2, :],
                            start=(j == 0), stop=(j == EO // 2 - 1),
                            perf_mode=DR,
                        )
                    nc.vector.tensor_copy(v8[:, t * (XT // 128) + st, :],
                                          ps[:, :M])
                sink(grpv)

        import collections
        filler = collections.deque()    # pair-slot work: PE DRs + DVE evacs
        fillerB = collections.deque()   # boundary work: ACT-evac out-proj

        def drain(budget=1):
            # budget in ~0.5us PE units: a proj half-group counts 1, an
            # out-proj tile 1, so draining never starves ACT of scores
            while filler and budget > 0:
                w, fn = filler.popleft()
                fn()
                budget -= w

        def drainB(n=1):
            # ACT-evac tiles slot into the q-tile boundary where the exp
            # stream pauses for the normalization chain anyway
            for _ in range(n):
                if fillerB:
                    fillerB.popleft()()
                elif filler:
                    w, fn = filler.popleft()
                    fn()

        def drain_all():
            while fillerB:
                fillerB.popleft()()
            while filler:
                filler.popleft()[1]()

        def attn_qtile(b, h, qt, rec01):
            qT8, kT8, v8, oT8 = qT8s[b], kT8s[b], v8s[b], oT8s[b]
            q_rhs = qT8[:, :, h, qt * QT:(qt + 1) * QT]
            ut = psU.tile([128, 512], F32, tag="ut")
            sums = psS.tile([16, 512], F32, tag="sm")
            npair = (qt + 1) * 2

            def flush(e2, pr):
                # ut/sums matmuls for pair pr, deferred two pairs so the
                # in-order PE isn't blocked on exp / the psU-psS turnaround
                nc.tensor.matmul(
                    ut[:], v8[:, 2 * pr:2 * pr + 2, h * D:(h + 1) * D], e2[:],
                    start=(pr == 0), stop=(pr == npair - 1), perf_mode=DR,
                )
                nc.tensor.matmul(
                    sums[:], on8_sb[:], e2[:],
                    start=(pr == 0), stop=(pr == npair - 1), perf_mode=DR,
                )

            pend = collections.deque()
            diag_flip[0] ^= 1
            for pr in range(npair):
                diag = pr - (npair - 2)  # 0 -> eA, 1 -> eB, <0 -> off-diag
                if diag < 0:
                    e2 = ep.tile([128, 2, 512], F8, tag="e")
                else:
                    e2 = eAs[diag_flip[0]] if diag == 0 else eBs[diag_flip[0]]
                sc = psC.tile([128, 2, 512], F32, tag="sc")
                for i in range(2):
                    kt = 2 * pr + i
                    jj = kt - qt * 4
                    lo = max(jj, 0) * 128
                    nc.tensor.matmul(
                        sc[:, i, lo:],
                        kT8[:, :, h, kt * 128:(kt + 1) * 128],
                        q_rhs[:, :, lo:],
                        start=True, stop=True, perf_mode=DR,
                    )
                if len(pend) >= 2:
                    flush(*pend.popleft())
                if diag < 0:
                    nc.scalar.activation(e2[:], sc[:], AF.Exp)
                else:
                    lo0 = 2 * diag * 128
                    nc.scalar.activation(e2[:, 0, lo0:], sc[:, 0, lo0:], AF.Exp)
                    nc.scalar.activation(e2[:, 1, lo0 + 128:],
                                         sc[:, 1, lo0 + 128:], AF.Exp)
                    # zero upper triangle of the two diagonal 128-blocks:
                    # block i covers cols [lo0+128i, lo0+128i+128); within
                    # the block keep col' >= partition
                    for i in range(2):
                        blk = e2[:, i, lo0 + 128 * i:lo0 + 128 * i + 128]
                        nc.gpsimd.affine_select(
                            blk, blk, [[1, 128]], ALU.is_ge, 0.0,
                            base=0, channel_multiplier=-1,
                        )
                pend.append((e2, pr))
                drain(2)
            while pend:
                flush(*pend.popleft())
            # evacuate ut to SBUF immediately so the single psU bank frees up
            # for the next (h, qt); normalization then runs off-critical-path
            nc.vector.reciprocal(rec01[:, h * 512:(h + 1) * 512],
                                 sums[0:1, :])
            ut_s = utp.tile([128, 512], F32, tag="uts")
            nc.vector.tensor_copy(ut_s[:], ut[:])
            return ut_s

        def attn_qt(b, qt):
            oT8 = oT8s[b]
            rec01 = rp.tile([1, 1024], F32, tag="rec")
            u0 = attn_qtile(b, 0, qt, rec01)
            drainB(2)
            u1 = attn_qtile(b, 1, qt, rec01)
            drainB(2)
            rb2 = rp.tile([128, 1024], F32, tag="rb")
            nc.gpsimd.partition_broadcast(rb2[:], rec01[:])
            nc.gpsimd.tensor_tensor(oT8[:, 0, qt * QT:(qt + 1) * QT], u0[:],
                                    rb2[:, 0:512], ALU.mult)
            nc.gpsimd.tensor_tensor(oT8[:, 1, qt * QT:(qt + 1) * QT], u1[:],
                                    rb2[:, 512:1024], ALU.mult)

        o_r = o.rearrange("b s (e2 f) -> b s e2 f", f=512)

        def out_proj_pairs(b, dve_only=False):
            """Yield one closure per double-width out-projection tile.

            Each closure runs two DR matmuls into a 2-bank psC tile, one
            double-width evac and one store.  With dve_only the evac always
            lands on DVE so it can ride inside another batch's attention
            phase without polluting the exp stream on ACT."""
            oT8 = oT8s[b]
            oTr = rds[b]["oTr"]
            for qi in range(S // 128):
                for et2 in range(E // 1024):
                    def pair_(qi=qi, et2=et2):
                        if dve_only:
                            # single-width tiles on psA: the attention owns
                            # psC, sharing it would serialize scores behind
                            # these evacs
                            for i in range(2):
                                et = et2 * 2 + i
                                ps = psA.tile([128, 512], F32, tag="pj")
                                if qi == 0:
                                    for h in range(HPC):
                                        nc.tensor.matmul(
                                            ps[:], oTr[:, h, :],
                                            wow_sb[:, h,
                                                   et * 512:(et + 1) * 512],
                                            start=(h == 0),
                                            stop=(h == HPC - 1),
                                        )
                                else:
                                    nc.tensor.matmul(
                                        ps[:],
                                        oT8[:, :, qi * 128:(qi + 1) * 128],
                                        wo8_sb[:, :, et * 512:(et + 1) * 512],
                                        start=True, stop=True, perf_mode=DR,
                                    )
                                osb = osp.tile([128, 512], BF16, tag="osb")
                                nc.vector.tensor_copy(osb[:], ps[:])
                                nc.sync.dma_start(
                                    o[b, qi * 128:(qi + 1) * 128,
                                      et * 512:(et + 1) * 512], osb[:])
                            return
                        ps2 = psC.tile([128, 2, 512], F32, tag="sc")
                        for i in range(2):
                            et = et2 * 2 + i
                            if qi == 0:
                                for h in range(HPC):
                                    nc.tensor.matmul(
                                        ps2[:, i, :], oTr[:, h, :],
                                        wow_sb[:, h, et * 512:(et + 1) * 512],
                                        start=(h == 0), stop=(h == HPC - 1),
                                    )
                            else:
                                nc.tensor.matmul(
                                    ps2[:, i, :],
                                    oT8[:, :, qi * 128:(qi + 1) * 128],
                                    wo8_sb[:, :, et * 512:(et + 1) * 512],
                                    start=True, stop=True, perf_mode=DR,
                                )
                        osb2 = osp.tile([128, 2, 512], BF16, tag="osb2")
                        k = qi * 2 + et2
                        if dve_only or k % 2 == 0:
                            nc.vector.tensor_copy(osb2[:], ps2[:])
                            nc.sync.dma_start(
                                o_r[b, qi * 128:(qi + 1) * 128,
                                    et2 * 2:et2 * 2 + 2], osb2[:])
                        else:
                            nc.scalar.copy(osb2[:], ps2[:])
                            nc.scalar.dma_start(
                                o_r[b, qi * 128:(qi + 1) * 128,
                                    et2 * 2:et2 * 2 + 2], osb2[:])
                    yield pair_

        # ---- schedule ----
        run_now = (lambda f: f())
        push = (lambda f: filler.append((1, f)))      # light (out-proj tile)
        push2 = (lambda f: filler.append((2, f)))     # heavy (proj group)
        # b0 attention q-tile qt only needs projection tiles <= qt, so
        # projections interleave between attention q-tiles: PE chews the next
        # proj tile while ACT drains the previous q-tile's exps.
        proj_tile(0, 0, b0_x[0], run_now)
        proj_tile(0, 1, b0_x[1], run_now)
        attn_qt(0, 0)
        redo_proj(0, run_now)
        redo_attn(0)
        attn_qt(0, 1)
        proj_tile(0, 2, b0_x[2], run_now)
        op0 = list(out_proj_pairs(0, dve_only=True))
        for fn in op0[:8]:
            filler.append((3, fn))
        attn_qt(0, 2)
        proj_tile(0, 3, b0_x[3], run_now)
        for fn in op0[8:16]:
            filler.append((3, fn))
        # prefetch b1 x-tiles on the SWDGE queue (Pool is quiet mid-run)
        b1_x = []
        for t in range(NT):
            x_t = xp.tile([128, EO, XT], F8, tag="x", name=f"x1_{t}")
            nc.gpsimd.dma_start(x_t[:], x8[1, t])
            b1_x.append(x_t)
        redo_proj(1, push2)
        proj_tile(1, 0, b1_x[0], push)
        proj_tile(1, 1, b1_x[1], push)
        attn_qt(0, 3)
        proj_tile(1, 2, b1_x[2], push)
        proj_tile(1, 3, b1_x[3], push)
        drain_all()
        redo_attn(1)
        # remaining b0 out-proj plus b1's rides inside the b1 attention
        # windows (b1 qt out-proj staggered one q-tile behind); the final
        # chunk is the only true tail
        op1 = list(out_proj_pairs(1, dve_only=True))
        op1_tail = list(out_proj_pairs(1))
        opq = op0[16:] + op1
        opq_alt = op0[16:] + op1_tail   # same tiles, engine-alternating
        served = 0
        for qt, hi in zip(range(NQT), (6, 14, 24, 38)):
            attn_qt(1, qt)
            # the chunk after the last q-tile trails the exp stream, so it
            # can use both evac engines
            src_list = opq if qt < NQT - 1 else opq_alt
            for fn in src_list[served:hi]:
                fn()
            served = hi
        drain_all()
        # tail: alternate both evac engines
        for fn in op1_tail[served - len(op0[16:]):]:
            fn()


_NC_CACHE = None


def _get_nc():
    global _NC_CACHE
    if _NC_CACHE is None:
        _NC_CACHE = build_nc()
    return _NC_CACHE


FP8 = ml_dtypes.float8_e4m3fn
BF = ml_dtypes.bfloat16


def _prep_inputs(x, Wq, bq, Wk, bk, Wv, bv, Wo, bo):
    x = np.asarray(x, dtype=np.float32)
    # [B, NT, 128, EO, XT] fp8 tiles of x^T
    x8 = np.ascontiguousarray(
        x.reshape(B, NT, XT, EO, 128).transpose(0, 1, 4, 3, 2)
    ).astype(FP8)
    # bf16 x^T slice for the first RED tokens of each batch
    xw = np.ascontiguousarray(
        x[:, :RED].reshape(B, RED, EO, 128).transpose(0, 3, 2, 1)
    ).astype(BF)
    on8 = np.ones((128, 2, 16), dtype=np.float32).astype(FP8)

    Wq = np.asarray(Wq, np.float32); Wk = np.asarray(Wk, np.float32)
    Wv = np.asarray(Wv, np.float32); Wo = np.asarray(Wo, np.float32)
    bqf = np.asarray(bq, np.float32); bkf = np.asarray(bk, np.float32)

    in_maps = []
    for c in range(NCORES):
        sl = slice(c * M, (c + 1) * M)

        def pack_w(wt):  # [E, M] -> [128, EO, M]
            return np.ascontiguousarray(
                wt.reshape(EO, 128, M).transpose(1, 0, 2))

        wq_t = pack_w(Wq[sl].T); wk_t = pack_w(Wk[sl].T)
        wv_t = pack_w(Wv[sl].T)
        wo_t = np.ascontiguousarray(
            Wo[:, sl].T.reshape(HPC, 128, E).transpose(1, 0, 2))
        in_maps.append({
            "x8": x8, "xw": xw, "on8": on8,
            "wq8": wq_t.astype(FP8), "wk8": wk_t.astype(FP8),
            "wv8": wv_t.astype(FP8), "wo8": wo_t.astype(FP8),
            "wqw": wq_t.astype(BF), "wkw": wk_t.astype(BF),
            "wvw": wv_t.astype(BF), "wow": wo_t.astype(BF),
            "bq": np.ascontiguousarray(
                (bqf[sl] * SCALE).reshape(HPC, 128).T),
            "bk": np.ascontiguousarray(bkf[sl].reshape(HPC, 128).T),
        })
    return in_maps


def run(inputs, trace=False):
    in_maps = _prep_inputs(
        inputs["x"], inputs["Wq"], inputs["bq"], inputs["Wk"], inputs["bk"],
        inputs["Wv"], inputs["bv"], inputs["Wo"], inputs["bo"],
    )
    nc = _get_nc()
    res = run_bass_kernel_spmd(nc, in_maps, list(range(NCORES)), trace=trace)
    acc = np.zeros((B, S, E), dtype=np.float64)
    for r in res.results:
        acc += r["o"].astype(np.float64)
    acc += np.asarray(inputs["bo"], dtype=np.float64)[None, None, :]
    acc += (np.asarray(inputs["bv"], dtype=np.float64)
            @ np.asarray(inputs["Wo"], dtype=np.float64).T)[None, None, :]
    return acc.astype(np.float32), res


def kernel(**inputs):
    out, _ = run(inputs, trace=False)
    return out
